# revision 2
# baseline (speedup 1.0000x reference)
"""TSSA causal self-attention Bass kernel, v2: head-split sharding.

Sharding: core i -> (batch b = i//2, head-half hh = i%2, i.e. heads
hh*8..hh*8+7 = w columns [hh*512:(hh+1)*512]).  Each core runs the FULL
T=4096 for its 8 heads, so the causal cumsums are core-local (carry chained
through the triangular matmul's last PSUM row -- no cross-core carry, no
one-hot chunk-sum matmuls, no prefix tables).

The only cross-core data dependency is the softmax-over-heads denominator:
rtot[t] = sum over all 16 heads of exp(tmp).  Each core AllReduce-adds its
local 8-head partial sums (two 8KB collectives, each fired 16 chunks before
its consumer -> fully hidden behind compute).

proj2 is column-split: each core computes a PARTIAL output
y_local @ (-Wp.T)[local 512 rows, :] over the full (T, C) and the host adds
the two partials per batch.  No y exchange (collectives are ~15us overhead +
40GB/s -- a 2MB exchange would cost ~67us).

Math per (b, local heads):
    w      = x @ Wa.T[:, loc]            # (T, 512) -> heads (T, 8, 64)
    wsq    = w * w
    denom  = cumsum_T(wsq) + 1e-12       # chained tri matmuls
    tmp    = seghsum_d(wsq / denom)      # fused DVE scan + boundary diff
    es     = exp(tmp)                    # (T, 8); rs = sum_h es
    Pi     = es / AllReduceAdd(rs)       # softmax over all 16 heads
    cumPi  = cumsum_T(Pi) + 1e-8         # f32 chain (exact)
    D      = cumsum_T(wsq*Pi) + cumPi    # cumA chain + cpe bcast via matmul
    y      = (w * Pi * cumPi) * (1/D)    # sign folded into wpTn
    out_p  = y @ (-Wp.T)[loc, :]         # partial; host adds the two halves
"""

import numpy as np
import ml_dtypes

B, T, C, H, D = 4, 4096, 1024, 16, 64
N_CORES = 8
P = 128
HL = H // 2          # 8 local heads
W = HL * D           # 512 local w columns
NCH = T // P         # 32 chunks

F32 = None
BF16 = None

_BUILD_CACHE = {}


def _ensure_scan_op():
    """Register a custom DVE op: inclusive prefix-scan of Src0*Src1 along the
    free dim.  Per-head sums then come from differencing the scan at head
    boundaries, fusing the (mul, segmented-reduce) pair into one DVE pass."""
    from concourse import dve_ops as dops
    if hasattr(dops, "_TT_MUL_SCAN_ANT"):
        return dops._TT_MUL_SCAN_ANT
    import numpy as np
    from concourse.dve_spec import Spec, Src0, Src1, AluOp, Bin, scan, lower, _has_src1
    from concourse.dve_uop import DveOpSpec

    def _ref(in0, in1, s0, s1, imm2):
        return np.cumsum(in0.astype(np.float32) * in1.astype(np.float32),
                         axis=-1, dtype=np.float32)

    spec = Spec(body=scan(AluOp.ADD, Bin(AluOp.MULTIPLY, Src0, Src1)),
                reference=_ref)
    shas = {}
    for ver in ("v3", "v4"):
        try:
            r = DveOpSpec(name="TT_MUL_SCAN_ANT", uops=lower(spec, ver=ver),
                          rd1_en=_has_src1(spec))
            shas[ver] = r.sha(ver)
        except Exception:
            pass
    op = dops.DveOp("TT_MUL_SCAN_ANT", spec, subdim=False, uops_sha=shas)
    dops.OPS.append(op)
    dops.CUSTOM_DVE_SPECS[op.name] = spec
    dops._SUB_OPCODE_FOR_NAME[op.name] = max(dops._SUB_OPCODE_FOR_NAME.values()) + 1
    dops._TT_MUL_SCAN_ANT = op
    return op


def _build(n_groups=4, fake_comm=False, use_ba=False, use_bp=False,
           use_tmpscale=False, f32out=False, reps=1,
           wcopy_eng="vector", yt_eng="scalar", pw_bufs=4, pdn_bufs=3,
           ost_eng="scalar", t1_eng="vector", pyt_bufs=2, po_bufs=2,
           cc_eng="gpsimd"):
    """Build the SPMD Bass program (symmetric across all cores)."""
    import concourse.bass as bass
    import concourse.bacc as bacc
    import concourse.mybir as mybir
    from concourse import tile

    dt = mybir.dt
    f32, bf16 = dt.float32, dt.bfloat16
    AF = mybir.ActivationFunctionType

    odt = f32 if f32out else bf16

    scan_op = _ensure_scan_op()

    nc = bacc.Bacc(None, target_bir_lowering=False, debug=False)

    # ---------------- I/O ----------------
    xT = nc.dram_tensor("xT", [C, T], bf16, kind="ExternalInput")
    waT = nc.dram_tensor("waT", [C, W], bf16, kind="ExternalInput")
    wpTn = nc.dram_tensor("wpTn", [W, C], bf16, kind="ExternalInput")
    ba_in = nc.dram_tensor("ba", [1, W], bf16, kind="ExternalInput")
    bp_in = nc.dram_tensor("bp", [1, C], bf16, kind="ExternalInput")
    tb_in = nc.dram_tensor("tb", [P, HL], f32, kind="ExternalInput")
    db_in = nc.dram_tensor("db64", [T, HL], f32, kind="ExternalInput")
    out = nc.dram_tensor("out", [T, C], odt, kind="ExternalOutput")

    # constants baked into the NEFF.  Token rows are REVERSED within each
    # 128-chunk (host pre-flips xT), so the causal cumsum is a LOWER-tri
    # matmul and the chunk total lands on PSUM partition 0 -- the carry
    # chain is then all partition-0 copies.
    lt_np = np.tril(np.ones((P, P), np.float32))
    utb_c = nc.inline_tensor(lt_np.astype(ml_dtypes.bfloat16), "utb_c")
    ut_c = nc.inline_tensor(lt_np, "ut_c")
    eye_np = np.eye(P, dtype=np.float32)
    eyeb_c = nc.inline_tensor(eye_np.astype(ml_dtypes.bfloat16), "eyeb_c")
    eye_c = nc.inline_tensor(eye_np, "eye_c")
    onesb_c = nc.inline_tensor(np.ones((1, P), ml_dtypes.bfloat16), "onesb_c")
    onesr_c = nc.inline_tensor(np.ones((1, P), np.float32), "onesr_c")
    # bmbx: rows 0..7 = head selector rows for the cpe-broadcast matmul,
    # rows 8..31 zero, row 32 = dynamic carrA row (32-aligned partition base
    # so the DVE chain-update may write it).
    bm_np = np.zeros((33, W), np.float32)
    for h in range(HL):
        bm_np[h, h * D:(h + 1) * D] = 1.0
    bmb_c = nc.inline_tensor(bm_np.astype(ml_dtypes.bfloat16), "bmb_c")
    # cpt: rows 0..7 = cpe^T (per chunk), row 32 = ones; zero-init the rest
    # (junk x 0 would still contaminate the matmul if it were NaN).
    cpt_np = np.zeros((33, P), np.float32)
    cpt_np[32, :] = 1.0
    cpt_c = nc.inline_tensor(cpt_np.astype(ml_dtypes.bfloat16), "cpt_c")
    carr0_c = nc.inline_tensor(
        np.full((1, W), 1e-12, ml_dtypes.bfloat16), "carr0_c")
    carrPi0_c = nc.inline_tensor(np.full((1, HL), 1e-8, np.float32),
                                 "carrPi0_c")

    # internal DRAM for the two softmax-denominator AllReduces
    cc1_in = nc.dram_tensor("cc1_in", [reps, P, NCH // 2], f32, kind="Internal")
    cc1_out = nc.dram_tensor("cc1_out", [reps, P, NCH // 2], f32,
                             kind="Internal")
    cc2_in = nc.dram_tensor("cc2_in", [reps, P, NCH // 2], f32, kind="Internal")
    cc2_out = nc.dram_tensor("cc2_out", [reps, P, NCH // 2], f32,
                             kind="Internal")
    rg = [[2 * g, 2 * g + 1] for g in range(n_groups)]

    with tile.TileContext(nc) as tc:
        with (
            tc.tile_pool(name="const", bufs=1) as cpool,
            tc.tile_pool(name="persist", bufs=1) as pp,
            tc.tile_pool(name="wmat", bufs=1) as wm,
        ):
            for rep in range(reps):
                # ---- phase-A-critical loads first ----
                wa_t = []
                for a in range(8):
                    t = wm.tile([P, W], bf16, tag=f"wa{a}")
                    nc.sync.dma_start(
                        t[:, :],
                        waT.ap().rearrange("(a p) n -> a p n", p=P)[a, :, :])
                    wa_t.append(t)
                utb_s = cpool.tile([P, P], bf16, tag="utb")
                nc.sync.dma_start(utb_s[:, :], utb_c.ap())
                onesb_s = cpool.tile([1, P], bf16, tag="onesb")
                nc.sync.dma_start(onesb_s[:, :], onesb_c.ap())
                ba_s = cpool.tile([1, W], bf16, tag="ba")
                if use_ba:
                    nc.sync.dma_start(ba_s[:, :], ba_in.ap())
                ut_s = cpool.tile([P, P], f32, tag="ut")
                eyeb_s = cpool.tile([P, P], bf16, tag="eyeb")
                eye_s = cpool.tile([P, P], f32, tag="eye")
                onesr_s = cpool.tile([1, P], f32, tag="onesr")
                bp_s = cpool.tile([1, C], bf16, tag="bp")
                tb_s = cpool.tile([P, HL], f32, tag="tb")
                db_s = cpool.tile([P, NCH, HL], f32, tag="db")

                def _late_const_loads():
                    nc.sync.dma_start(ut_s[:, :], ut_c.ap())
                    nc.sync.dma_start(eyeb_s[:, :], eyeb_c.ap())
                    nc.sync.dma_start(eye_s[:, :], eye_c.ap())
                    nc.sync.dma_start(onesr_s[:, :], onesr_c.ap())
                    if use_bp:
                        nc.sync.dma_start(bp_s[:, :], bp_in.ap())
                    if use_tmpscale:
                        nc.sync.dma_start(tb_s[:, :], tb_in.ap())
                        nc.sync.dma_start(
                            db_s[:, :, :],
                            db_in.ap().rearrange("(j p) h -> p j h", p=P))

                # ---- persistent stores ----
                w_st = pp.tile([P, NCH, W], bf16, tag="w_st")
                sqb_st = pp.tile([P, NCH, W], bf16, tag="sqb_st")
                pi_st = pp.tile([P, NCH, HL], f32, tag="pi_st")   # holds es
                piF = pp.tile([P, NCH, HL], f32, tag="piF")
                rs_st = pp.tile([P, NCH], f32, tag="rs_st")
                rt_st = pp.tile([P, NCH], f32, tag="rt_st")
                rr_st = pp.tile([P, NCH], f32, tag="rr_st")
                carrb = pp.tile([1, NCH + 1, W], bf16, tag="carrb")
                carrPi = pp.tile([1, NCH + 1, HL], f32, tag="carrPi")
                bmbx2, cpt2 = [], []
                for p_ in range(2):
                    bmbx_p = pp.tile([33, W], bf16, tag=f"bmbx{p_}")
                    bmbx2.append(bmbx_p)
                    cpt_p = pp.tile([33, P], bf16, tag=f"cpt{p_}")
                    cpt2.append(cpt_p)

                def _late_table_loads():
                    nc.sync.dma_start(carrb[0:1, 0, :], carr0_c.ap())
                    nc.sync.dma_start(carrPi[0:1, 0, :], carrPi0_c.ap())
                    for p_ in range(2):
                        nc.sync.dma_start(bmbx2[p_][:, :], bmb_c.ap())
                        nc.sync.dma_start(cpt2[p_][:, :], cpt_c.ap())

                xT_r = xT.ap().rearrange("(a p) t -> p a t", p=P)

                # ================= phase A: proj1, denom, softmax numerator ==
                with (
                    tc.tile_pool(name="xt", bufs=3) as xt_pool,
                    tc.tile_pool(name="rdA", bufs=2) as rd_pool,
                    tc.tile_pool(name="wnA", bufs=2) as wn_pool,
                    tc.tile_pool(name="tinyA", bufs=3) as tiny_pool,
                    tc.tile_pool(name="pw", bufs=pw_bufs,
                                 space="PSUM") as pw_pool,
                    tc.tile_pool(name="pdn", bufs=pdn_bufs,
                                 space="PSUM") as pdn_pool,
                ):
                    def _a_tail(j):
                        """Deferred denom/softmax stage for chunk j (emitted
                        behind chunk j+1's proj1 so the PE never waits on the
                        ACT square)."""
                        pdn = pdn_pool.tile([P, W], f32, tag="pdn")
                        nc.tensor.matmul(pdn[:, :], utb_s[:, :], sqb_st[:, j, :],
                                         start=True, stop=False)
                        nc.tensor.matmul(pdn[:, :], onesb_s[0:1, :],
                                         carrb[0:1, j, :], start=False,
                                         stop=True)
                        nc.scalar.copy(carrb[0:1, j + 1, :], pdn[0:1, :])
                        rd = rd_pool.tile([P, W], f32, tag="rdA")
                        nc.vector.reciprocal_approx_fast(rd[:, :], pdn[:, :])
                        wn = wn_pool.tile([P, W], f32, tag="wnA")
                        nc.vector._custom_dve(scan_op, out=wn[:, :],
                                              in0=sqb_st[:, j, :], in1=rd[:, :])
                        wn3 = wn[:, :].rearrange("p (h d) -> p h d", d=D)
                        red = tiny_pool.tile([P, HL], f32, tag="red")
                        nc.vector.tensor_copy(red[:, 0:1], wn3[:, 0:1, D - 1])
                        nc.vector.tensor_sub(
                            red[:, 1:HL].rearrange("p (h o) -> p h o", o=1),
                            wn3[:, 1:HL, D - 1], wn3[:, 0:HL - 1, D - 1])
                        if use_tmpscale:
                            t1 = tiny_pool.tile([P, HL], f32, tag="t1")
                            nc.vector.tensor_add(t1[:, :], red[:, :],
                                                 db_s[:, j, :])
                            tmpj = tiny_pool.tile([P, HL], f32, tag="tmpj")
                            nc.vector.tensor_mul(tmpj[:, :], t1[:, :], tb_s[:, :])
                        else:
                            tmpj = red
                        # tmp <= D*temp = 64: exp fits f32, skip max-sub
                        nc.scalar.activation(pi_st[:, j, :], tmpj[:, :], AF.Exp,
                                             accum_out=rs_st[:, j:j + 1])

                    def _xt_load(j):
                        xt = xt_pool.tile([P, 8, P], bf16, tag="xt")
                        nc.gpsimd.dma_start(xt[:, :, :],
                                            xT_r[:, :, j * P:(j + 1) * P])
                        return xt

                    xtq = [_xt_load(j) for j in range(3)]
                    for j in range(NCH):
                        xt = xtq.pop(0)
                        if j + 3 < NCH:
                            xtq.append(_xt_load(j + 3))
                        pw = pw_pool.tile([P, W], f32, tag="pw")
                        for a in range(8):
                            nc.tensor.matmul(pw[:, :], xt[:, a, :], wa_t[a][:, :],
                                             start=(a == 0),
                                             stop=(a == 7 and not use_ba))
                        if use_ba:
                            nc.tensor.matmul(pw[:, :], onesb_s[0:1, :],
                                             ba_s[0:1, :], start=False,
                                             stop=True)
                        nc.scalar.activation(sqb_st[:, j, :], pw[:, :],
                                             AF.Square)
                        if wcopy_eng == "scalar":
                            nc.scalar.copy(w_st[:, j, :], pw[:, :])
                        else:
                            nc.vector.tensor_copy(w_st[:, j, :], pw[:, :])
                        if j == 0:
                            _late_table_loads()
                            _late_const_loads()
                        if j > 0:
                            _a_tail(j - 1)
                        if j == NCH // 2:
                            _fire_ar(nc, tc, rep, rg, fake_comm, cc1_in,
                                     cc1_out, rs_st, rt_st, rr_st,
                                     pi_st, piF, 0, cc_eng)
                        if j == 2:
                            # prefetch proj2 weights (only needed in phase B)
                            wp_t = []
                            for a in range(4):
                                t = wm.tile([P, C], bf16, tag=f"wp{a}")
                                nc.sync.dma_start(
                                    t[:, :],
                                    wpTn.ap().rearrange("(a p) n -> a p n",
                                                        p=P)[a, :, :])
                                wp_t.append(t)
                    _a_tail(NCH - 1)
                    _fire_ar(nc, tc, rep, rg, fake_comm, cc2_in, cc2_out,
                             rs_st, rt_st, rr_st, pi_st, piF, 1, cc_eng)

                # ================= phase B: Pi, dots, y, partial proj2 =======
                with (
                    tc.tile_pool(name="piB", bufs=3) as pi_pool,
                    tc.tile_pool(name="wspB", bufs=3) as wsp_pool,
                    tc.tile_pool(name="cpeB", bufs=3) as cpe_pool,
                    tc.tile_pool(name="rdB", bufs=2) as rd3_pool,
                    tc.tile_pool(name="t1B", bufs=2) as t1_pool,
                    tc.tile_pool(name="yB", bufs=3) as y_pool,
                    tc.tile_pool(name="ytB", bufs=3) as yt_pool,
                    tc.tile_pool(name="ostB", bufs=3) as ost_pool,
                    tc.tile_pool(name="tinyB", bufs=3) as tinyB_pool,
                    tc.tile_pool(name="psp", bufs=2, space="PSUM") as psp_pool,
                    tc.tile_pool(name="pda", bufs=2, space="PSUM") as pda_pool,
                    tc.tile_pool(name="pyt", bufs=pyt_bufs, space="PSUM") as pyt_pool,
                    tc.tile_pool(name="po", bufs=po_bufs, space="PSUM") as po_pool,
                ):
                    ydeq = []

                    def _b_trans(j, y):
                        """Transpose chunk j's y (deferred two iterations so
                        the DVE y-chain has slack)."""
                        pyt = pyt_pool.tile([P, W], bf16, tag="pyt")
                        for i in range(4):
                            nc.tensor.transpose(pyt[:, i * P:(i + 1) * P],
                                                y[:, i * P:(i + 1) * P],
                                                eyeb_s[:, :])
                        yt = yt_pool.tile([P, 4, P], bf16, tag="yt")
                        yt_e = (nc.scalar.copy if yt_eng == "scalar"
                                else nc.vector.tensor_copy)
                        yt_e(yt[:, :, :],
                             pyt[:, :].rearrange("p (a q) -> p a q", q=P))
                        return yt

                    def _b_proj(j, yt):
                        """Partial proj2 for chunk j."""
                        for hh in range(2):
                            po = po_pool.tile([P, 512], f32, tag="po")
                            for a in range(4):
                                nc.tensor.matmul(
                                    po[:, :], yt[:, a, :],
                                    wp_t[a][:, hh * 512:(hh + 1) * 512],
                                    start=(a == 0),
                                    stop=(a == 3 and not use_bp))
                            if use_bp:
                                nc.tensor.matmul(
                                    po[:, :], onesb_s[0:1, :],
                                    bp_s[0:1, hh * 512:(hh + 1) * 512],
                                    start=False, stop=True)
                            ost = ost_pool.tile([P, 512], odt, tag="ost")
                            oe = ost_eng
                            if ost_eng == "split":
                                oe = "vector" if hh == 0 else "scalar"
                            elif ost_eng == "split2":
                                oe = "scalar" if hh == 0 else "vector"
                            if oe == "scalar":
                                nc.scalar.copy(ost[:, :], po[:, :])
                            else:
                                nc.vector.tensor_copy(ost[:, :], po[:, :])
                            nc.sync.dma_start(
                                out.ap()[j * P:(j + 1) * P,
                                         hh * 512:(hh + 1) * 512], ost[:, :])

                    def _b_mid(j, st):
                        """Deferred dots/y stage for chunk j (one iteration of
                        slack for the cpe->cpt chain)."""
                        pi, wsp, cpe = st
                        cpt = cpt2[j % 2]
                        bmbx_r = bmbx2[j % 2]
                        bmbx_w = bmbx2[(j + 1) % 2]
                        # D = cumA + carrA + bcast(cumPi): tri + one matmul
                        pda = pda_pool.tile([P, W], f32, tag="pda")
                        nc.tensor.matmul(pda[:, :], utb_s[:, :], wsp[:, :],
                                         start=True, stop=False)
                        nc.tensor.matmul(pda[:, :], cpt[:, :], bmbx_r[:, :],
                                         start=False, stop=True)
                        # next carrA row = total row (p0) minus its cpe part
                        nc.vector.tensor_sub(
                            bmbx_w[32:33, :].rearrange("o (h d) -> o h d", d=D),
                            pda[0:1, :].rearrange("o (h d) -> o h d", d=D),
                            cpe[0:1, :].rearrange("o (h u) -> o h u", u=1)
                            .to_broadcast((1, HL, D)))
                        rd3 = rd3_pool.tile([P, W], f32, tag="rd3")
                        nc.vector.reciprocal_approx_fast(rd3[:, :], pda[:, :])
                        g = tinyB_pool.tile([P, HL], f32, tag="g")
                        nc.vector.tensor_mul(g[:, :], pi, cpe[:, :])
                        t1 = t1_pool.tile([P, W], f32, tag="t1f")
                        t1_e = (nc.gpsimd.tensor_mul if t1_eng == "gpsimd"
                                else nc.vector.tensor_mul)
                        t1_e(t1[:, :], w_st[:, j, :], rd3[:, :])
                        y = y_pool.tile([P, W], bf16, tag="ybf")
                        nc.vector.tensor_mul(
                            y[:, :].rearrange("p (h d) -> p h d", d=D),
                            t1[:, :].rearrange("p (h d) -> p h d", d=D),
                            g[:, :].rearrange("p (h o) -> p h o", o=1)
                            .to_broadcast((P, HL, D)))
                        ydeq.append((j, y))

                    stq = []
                    ytq = []
                    for j in range(NCH):
                        if len(ydeq) == 2:
                            jt, yd = ydeq.pop(0)
                            ytq.append((jt, _b_trans(jt, yd)))
                        if len(ytq) == 2:
                            jd, ytd = ytq.pop(0)
                        else:
                            jd = None
                        pi = piF[:, j, :]
                        wsp = wsp_pool.tile([P, W], bf16, tag="wsp")
                        nc.gpsimd.tensor_mul(
                            wsp[:, :].rearrange("p (h d) -> p h d", d=D),
                            sqb_st[:, j, :].rearrange("p (h d) -> p h d", d=D),
                            pi.rearrange("p (h o) -> p h o", o=1)
                            .to_broadcast((P, HL, D)))
                        # cumPi chain (f32, exact); spare psum cols hold the
                        # transposed cpe for the cpe-broadcast matmul.
                        psp = psp_pool.tile([P, 256], f32, tag="psp")
                        nc.tensor.matmul(psp[:, 0:HL], ut_s[:, :], pi,
                                         start=True, stop=False)
                        nc.tensor.matmul(psp[:, 0:HL], onesr_s[0:1, :],
                                         carrPi[0:1, j, :], start=False,
                                         stop=True)
                        nc.scalar.copy(carrPi[0:1, j + 1, :],
                                       psp[0:1, 0:HL])
                        cpe = cpe_pool.tile([P, HL], f32, tag="cpe")
                        nc.scalar.copy(cpe[:, :], psp[:, 0:HL])
                        if jd is not None:
                            _b_proj(jd, ytd)
                        if stq:
                            _b_mid(*stq.pop(0))
                        nc.tensor.transpose(psp[0:HL, 128:256], cpe[:, :],
                                            eye_s[:, :])
                        nc.vector.tensor_copy(cpt2[j % 2][0:HL, :],
                                              psp[0:HL, 128:256])
                        stq.append((j, (pi, wsp, cpe)))
                    _b_mid(*stq.pop(0))
                    for jt, yd in ydeq:
                        ytq.append((jt, _b_trans(jt, yd)))
                    for jd, ytd in ytq:
                        _b_proj(jd, ytd)

    nc.finalize()
    return nc


def _fire_ar(nc, tc, rep, rg, fake_comm, cc_in, cc_out, rs_st, rt_st, rr_st,
             pi_st, piF, half, cc_eng="gpsimd"):
    """Stage local 8-head exp-sums for 16 chunks, AllReduce-add with the
    partner core, read back the 16-head totals and take reciprocals."""
    import concourse.mybir as mybir
    n = NCH // 2
    lo = half * n
    nc.sync.dma_start(cc_in.ap()[rep], rs_st[:, lo:lo + n])
    if fake_comm:
        nc.sync.dma_start(cc_out.ap()[rep], cc_in.ap()[rep])
    else:
        getattr(nc, cc_eng).collective_compute(
            "AllReduce", mybir.AluOpType.add, replica_groups=rg,
            ins=[cc_in.ap()[rep].opt()], outs=[cc_out.ap()[rep].opt()])
    nc.sync.dma_start(rt_st[:, lo:lo + n], cc_out.ap()[rep])
    nc.vector.reciprocal_approx_fast(rr_st[:, lo:lo + n], rt_st[:, lo:lo + n])
    # batch-normalize: Pi = es * (1/rtot) for all 16 chunks in one DVE op
    nc.vector.tensor_mul(
        piF[:, lo:lo + n, :], pi_st[:, lo:lo + n, :],
        rr_st[:, lo:lo + n].rearrange("p (c o) -> p c o", o=1)
        .to_broadcast((P, n, HL)))


def _get_nc(**kw):
    key = tuple(sorted(kw.items()))
    if key not in _BUILD_CACHE:
        _BUILD_CACHE[key] = _build(**kw)
    return _BUILD_CACHE[key]


def make_in_maps(x, Wa, ba, Wp, bp, temp, denom_bias):
    """Host-side sharding: core i -> (b=i//2, head-half=i%2)."""
    bf = ml_dtypes.bfloat16
    waT = np.ascontiguousarray(Wa.T).astype(bf)          # [C, C]
    wpTn = np.ascontiguousarray(-Wp.T).astype(bf)        # [C, C]
    # token rows reversed within each 128-chunk (see _build)
    xTs = []
    for b in range(B):
        xr = x[b].reshape(NCH, P, C)[:, ::-1, :].reshape(T, C)
        xTs.append(np.ascontiguousarray(xr.T).astype(bf))
    in_maps = []
    for i in range(N_CORES):
        b, hh = i // 2, i % 2
        wa_loc = np.ascontiguousarray(waT[:, hh * W:(hh + 1) * W])
        wp_loc = np.ascontiguousarray(wpTn[hh * W:(hh + 1) * W, :])
        ba_loc = np.ascontiguousarray(
            ba[hh * W:(hh + 1) * W].reshape(1, W)).astype(bf)
        bp_half = np.ascontiguousarray((bp / 2.0).reshape(1, C)).astype(bf)
        tb = np.ascontiguousarray(np.broadcast_to(
            temp[hh * HL:(hh + 1) * HL].reshape(1, HL), (P, HL))
        ).astype(np.float32)
        dbr = (D * denom_bias[hh * HL:(hh + 1) * HL, :, 0].T)
        dbr = dbr.reshape(NCH, P, HL)[:, ::-1, :].reshape(T, HL)
        db64 = np.ascontiguousarray(dbr).astype(np.float32)
        in_maps.append({
            "xT": xTs[b], "waT": wa_loc, "wpTn": wp_loc, "ba": ba_loc,
            "bp": bp_half, "tb": tb, "db64": db64,
        })
    return in_maps


def kernel(x, Wa, ba, Wp, bp, temp, denom_bias):
    x = np.asarray(x)
    use_ba = bool(np.any(np.asarray(ba)))
    use_bp = bool(np.any(np.asarray(bp)))
    use_tmpscale = bool(np.any(np.asarray(denom_bias))
                        or not np.all(np.asarray(temp) == 1.0))
    nc = _get_nc(use_ba=use_ba, use_bp=use_bp, use_tmpscale=use_tmpscale)
    in_maps = make_in_maps(np.asarray(x), np.asarray(Wa), np.asarray(ba),
                           np.asarray(Wp), np.asarray(bp), np.asarray(temp),
                           np.asarray(denom_bias))
    from concourse import bass_utils
    res = bass_utils.run_bass_kernel_spmd(nc, in_maps,
                                          core_ids=list(range(N_CORES)))
    out = np.empty((B, T, C), np.float32)
    for b in range(B):
        s = (res.results[2 * b]["out"].astype(np.float32)
             + res.results[2 * b + 1]["out"].astype(np.float32))
        out[b] = s.reshape(NCH, P, C)[:, ::-1, :].reshape(T, C)
    return out


# revision 3
# speedup vs baseline: 1.0107x; 1.0107x over previous
"""TSSA causal self-attention Bass kernel, v2: head-split sharding.

Sharding: core i -> (batch b = i//2, head-half hh = i%2, i.e. heads
hh*8..hh*8+7 = w columns [hh*512:(hh+1)*512]).  Each core runs the FULL
T=4096 for its 8 heads, so the causal cumsums are core-local (carry chained
through the triangular matmul's last PSUM row -- no cross-core carry, no
one-hot chunk-sum matmuls, no prefix tables).

The only cross-core data dependency is the softmax-over-heads denominator:
rtot[t] = sum over all 16 heads of exp(tmp).  Each core AllReduce-adds its
local 8-head partial sums (two 8KB collectives, each fired 16 chunks before
its consumer -> fully hidden behind compute).

proj2 is column-split: each core computes a PARTIAL output
y_local @ (-Wp.T)[local 512 rows, :] over the full (T, C) and the host adds
the two partials per batch.  No y exchange (collectives are ~15us overhead +
40GB/s -- a 2MB exchange would cost ~67us).

Math per (b, local heads):
    w      = x @ Wa.T[:, loc]            # (T, 512) -> heads (T, 8, 64)
    wsq    = w * w
    denom  = cumsum_T(wsq) + 1e-12       # chained tri matmuls
    tmp    = seghsum_d(wsq / denom)      # fused DVE scan + boundary diff
    es     = exp(tmp)                    # (T, 8); rs = sum_h es
    Pi     = es / AllReduceAdd(rs)       # softmax over all 16 heads
    cumPi  = cumsum_T(Pi) + 1e-8         # f32 chain (exact)
    D      = cumsum_T(wsq*Pi) + cumPi    # cumA chain + cpe bcast via matmul
    y      = (w * Pi * cumPi) * (1/D)    # sign folded into wpTn
    out_p  = y @ (-Wp.T)[loc, :]         # partial; host adds the two halves
"""

import numpy as np
import ml_dtypes

B, T, C, H, D = 4, 4096, 1024, 16, 64
N_CORES = 8
P = 128
HL = H // 2          # 8 local heads
W = HL * D           # 512 local w columns
NCH = T // P         # 32 chunks

F32 = None
BF16 = None

_BUILD_CACHE = {}


def _ensure_scan_op():
    """Register a custom DVE op: inclusive prefix-scan of Src0*Src1 along the
    free dim.  Per-head sums then come from differencing the scan at head
    boundaries, fusing the (mul, segmented-reduce) pair into one DVE pass."""
    from concourse import dve_ops as dops
    if hasattr(dops, "_TT_MUL_SCAN_ANT"):
        return dops._TT_MUL_SCAN_ANT
    import numpy as np
    from concourse.dve_spec import Spec, Src0, Src1, AluOp, Bin, scan, lower, _has_src1
    from concourse.dve_uop import DveOpSpec

    def _ref(in0, in1, s0, s1, imm2):
        return np.cumsum(in0.astype(np.float32) * in1.astype(np.float32),
                         axis=-1, dtype=np.float32)

    spec = Spec(body=scan(AluOp.ADD, Bin(AluOp.MULTIPLY, Src0, Src1)),
                reference=_ref)
    shas = {}
    for ver in ("v3", "v4"):
        try:
            r = DveOpSpec(name="TT_MUL_SCAN_ANT", uops=lower(spec, ver=ver),
                          rd1_en=_has_src1(spec))
            shas[ver] = r.sha(ver)
        except Exception:
            pass
    op = dops.DveOp("TT_MUL_SCAN_ANT", spec, subdim=False, uops_sha=shas)
    dops.OPS.append(op)
    dops.CUSTOM_DVE_SPECS[op.name] = spec
    dops._SUB_OPCODE_FOR_NAME[op.name] = max(dops._SUB_OPCODE_FOR_NAME.values()) + 1
    dops._TT_MUL_SCAN_ANT = op
    return op


def _build(n_groups=4, fake_comm=False, use_ba=False, use_bp=False,
           use_tmpscale=False, f32out=False, reps=1,
           wcopy_eng="vector", yt_eng="scalar", pw_bufs=4, pdn_bufs=3,
           ost_eng="scalar", t1_eng="vector", pyt_bufs=2, po_bufs=2,
           cc_eng="gpsimd"):
    """Build the SPMD Bass program (symmetric across all cores)."""
    import concourse.bass as bass
    import concourse.bacc as bacc
    import concourse.mybir as mybir
    from concourse import tile

    dt = mybir.dt
    f32, bf16 = dt.float32, dt.bfloat16
    AF = mybir.ActivationFunctionType

    odt = f32 if f32out else bf16

    scan_op = _ensure_scan_op()

    nc = bacc.Bacc(None, target_bir_lowering=False, debug=False)

    # ---------------- I/O ----------------
    xT = nc.dram_tensor("xT", [C, T], bf16, kind="ExternalInput")
    waT = nc.dram_tensor("waT", [C, W], bf16, kind="ExternalInput")
    wpTn = nc.dram_tensor("wpTn", [W, C], bf16, kind="ExternalInput")
    ba_in = nc.dram_tensor("ba", [1, W], bf16, kind="ExternalInput")
    bp_in = nc.dram_tensor("bp", [1, C], bf16, kind="ExternalInput")
    tb_in = nc.dram_tensor("tb", [P, HL], f32, kind="ExternalInput")
    db_in = nc.dram_tensor("db64", [T, HL], f32, kind="ExternalInput")
    out = nc.dram_tensor("out", [T, C], odt, kind="ExternalOutput")

    # constants baked into the NEFF.  Token rows are REVERSED within each
    # 128-chunk (host pre-flips xT), so the causal cumsum is a LOWER-tri
    # matmul and the chunk total lands on PSUM partition 0 -- the carry
    # chain is then all partition-0 copies.
    lt_np = np.tril(np.ones((P, P), np.float32))
    utb_c = nc.inline_tensor(lt_np.astype(ml_dtypes.bfloat16), "utb_c")
    ut_c = nc.inline_tensor(lt_np, "ut_c")
    eye_np = np.eye(P, dtype=np.float32)
    eyeb_c = nc.inline_tensor(eye_np.astype(ml_dtypes.bfloat16), "eyeb_c")
    eye_c = nc.inline_tensor(eye_np, "eye_c")
    onesb_c = nc.inline_tensor(np.ones((1, P), ml_dtypes.bfloat16), "onesb_c")
    onesr_c = nc.inline_tensor(np.ones((1, P), np.float32), "onesr_c")
    # bmbx: rows 0..7 = head selector rows for the cpe-broadcast matmul,
    # rows 8..31 zero, row 32 = dynamic carrA row (32-aligned partition base
    # so the DVE chain-update may write it).
    bm_np = np.zeros((33, W), np.float32)
    for h in range(HL):
        bm_np[h, h * D:(h + 1) * D] = 1.0
    bmb_c = nc.inline_tensor(bm_np.astype(ml_dtypes.bfloat16), "bmb_c")
    # cpt: rows 0..7 = cpe^T (per chunk), row 32 = ones; zero-init the rest
    # (junk x 0 would still contaminate the matmul if it were NaN).
    cpt_np = np.zeros((33, P), np.float32)
    cpt_np[32, :] = 1.0
    cpt_c = nc.inline_tensor(cpt_np.astype(ml_dtypes.bfloat16), "cpt_c")
    carr0_c = nc.inline_tensor(
        np.full((1, W), 1e-12, ml_dtypes.bfloat16), "carr0_c")
    carrPi0_c = nc.inline_tensor(np.full((1, HL), 1e-8, np.float32),
                                 "carrPi0_c")

    # internal DRAM for the two softmax-denominator AllReduces
    cc1_in = nc.dram_tensor("cc1_in", [reps, P, NCH // 2], f32, kind="Internal")
    cc1_out = nc.dram_tensor("cc1_out", [reps, P, NCH // 2], f32,
                             kind="Internal")
    cc2_in = nc.dram_tensor("cc2_in", [reps, P, NCH // 2], f32, kind="Internal")
    cc2_out = nc.dram_tensor("cc2_out", [reps, P, NCH // 2], f32,
                             kind="Internal")
    rg = [[2 * g, 2 * g + 1] for g in range(n_groups)]

    with tile.TileContext(nc) as tc:
        with (
            tc.tile_pool(name="const", bufs=1) as cpool,
            tc.tile_pool(name="persist", bufs=1) as pp,
            tc.tile_pool(name="wmat", bufs=1) as wm,
        ):
            for rep in range(reps):
                # ---- phase-A-critical loads first ----
                wa_t = []
                for a in range(8):
                    t = wm.tile([P, W], bf16, tag=f"wa{a}")
                    nc.sync.dma_start(
                        t[:, :],
                        waT.ap().rearrange("(a p) n -> a p n", p=P)[a, :, :])
                    wa_t.append(t)
                utb_s = cpool.tile([P, P], bf16, tag="utb")
                nc.sync.dma_start(utb_s[:, :], utb_c.ap())
                onesb_s = cpool.tile([1, P], bf16, tag="onesb")
                nc.sync.dma_start(onesb_s[:, :], onesb_c.ap())
                ba_s = cpool.tile([1, W], bf16, tag="ba")
                if use_ba:
                    nc.sync.dma_start(ba_s[:, :], ba_in.ap())
                ut_s = cpool.tile([P, P], f32, tag="ut")
                eyeb_s = cpool.tile([P, P], bf16, tag="eyeb")
                eye_s = cpool.tile([P, P], f32, tag="eye")
                onesr_s = cpool.tile([1, P], f32, tag="onesr")
                bp_s = cpool.tile([1, C], bf16, tag="bp")
                tb_s = cpool.tile([P, HL], f32, tag="tb")
                db_s = cpool.tile([P, NCH, HL], f32, tag="db")

                def _late_const_loads():
                    nc.sync.dma_start(ut_s[:, :], ut_c.ap())
                    nc.sync.dma_start(eyeb_s[:, :], eyeb_c.ap())
                    nc.sync.dma_start(eye_s[:, :], eye_c.ap())
                    nc.sync.dma_start(onesr_s[:, :], onesr_c.ap())
                    if use_bp:
                        nc.sync.dma_start(bp_s[:, :], bp_in.ap())
                    if use_tmpscale:
                        nc.sync.dma_start(tb_s[:, :], tb_in.ap())
                        nc.sync.dma_start(
                            db_s[:, :, :],
                            db_in.ap().rearrange("(j p) h -> p j h", p=P))

                # ---- persistent stores ----
                w_st = pp.tile([P, NCH, W], bf16, tag="w_st")
                sqb_st = pp.tile([P, NCH, W], bf16, tag="sqb_st")
                pi_st = pp.tile([P, NCH, HL], f32, tag="pi_st")   # holds es
                piF = pp.tile([P, NCH, HL], f32, tag="piF")
                rs_st = pp.tile([P, NCH], f32, tag="rs_st")
                rt_st = pp.tile([P, NCH], f32, tag="rt_st")
                rr_st = pp.tile([P, NCH], f32, tag="rr_st")
                carrb = pp.tile([1, NCH + 1, W], bf16, tag="carrb")
                carrPi = pp.tile([1, NCH + 1, HL], f32, tag="carrPi")
                bmbx2, cpt2 = [], []
                for p_ in range(2):
                    bmbx_p = pp.tile([33, W], bf16, tag=f"bmbx{p_}")
                    bmbx2.append(bmbx_p)
                    cpt_p = pp.tile([33, P], bf16, tag=f"cpt{p_}")
                    cpt2.append(cpt_p)

                def _late_table_loads():
                    nc.sync.dma_start(carrb[0:1, 0, :], carr0_c.ap())
                    nc.sync.dma_start(carrPi[0:1, 0, :], carrPi0_c.ap())
                    for p_ in range(2):
                        nc.sync.dma_start(bmbx2[p_][:, :], bmb_c.ap())
                        nc.sync.dma_start(cpt2[p_][:, :], cpt_c.ap())

                xT_r = xT.ap().rearrange("(a p) t -> p a t", p=P)

                # ================= phase A: proj1, denom, softmax numerator ==
                with (
                    tc.tile_pool(name="xt", bufs=3) as xt_pool,
                    tc.tile_pool(name="rdA", bufs=2) as rd_pool,
                    tc.tile_pool(name="wnA", bufs=2) as wn_pool,
                    tc.tile_pool(name="tinyA", bufs=3) as tiny_pool,
                    tc.tile_pool(name="pw", bufs=pw_bufs,
                                 space="PSUM") as pw_pool,
                    tc.tile_pool(name="pdn", bufs=pdn_bufs,
                                 space="PSUM") as pdn_pool,
                ):
                    def _a_tail(j):
                        """Deferred denom/softmax stage for chunk j (emitted
                        behind chunk j+1's proj1 so the PE never waits on the
                        ACT square)."""
                        pdn = pdn_pool.tile([P, W], f32, tag="pdn")
                        nc.tensor.matmul(pdn[:, :], utb_s[:, :], sqb_st[:, j, :],
                                         start=True, stop=False)
                        nc.tensor.matmul(pdn[:, :], onesb_s[0:1, :],
                                         carrb[0:1, j, :], start=False,
                                         stop=True)
                        nc.scalar.copy(carrb[0:1, j + 1, :], pdn[0:1, :])
                        rd = rd_pool.tile([P, W], f32, tag="rdA")
                        nc.vector.reciprocal_approx_fast(rd[:, :], pdn[:, :])
                        wn = wn_pool.tile([P, W], f32, tag="wnA")
                        nc.vector._custom_dve(scan_op, out=wn[:, :],
                                              in0=sqb_st[:, j, :], in1=rd[:, :])
                        wn3 = wn[:, :].rearrange("p (h d) -> p h d", d=D)
                        red = tiny_pool.tile([P, HL], f32, tag="red")
                        nc.vector.tensor_copy(red[:, 0:1], wn3[:, 0:1, D - 1])
                        nc.vector.tensor_sub(
                            red[:, 1:HL].rearrange("p (h o) -> p h o", o=1),
                            wn3[:, 1:HL, D - 1], wn3[:, 0:HL - 1, D - 1])
                        if use_tmpscale:
                            t1 = tiny_pool.tile([P, HL], f32, tag="t1")
                            nc.vector.tensor_add(t1[:, :], red[:, :],
                                                 db_s[:, j, :])
                            tmpj = tiny_pool.tile([P, HL], f32, tag="tmpj")
                            nc.vector.tensor_mul(tmpj[:, :], t1[:, :], tb_s[:, :])
                        else:
                            tmpj = red
                        # tmp <= D*temp = 64: exp fits f32, skip max-sub
                        nc.scalar.activation(pi_st[:, j, :], tmpj[:, :], AF.Exp,
                                             accum_out=rs_st[:, j:j + 1])

                    def _xt_load(j, q):
                        xt = xt_pool.tile([P, 8, P], bf16, tag="xt")
                        q.dma_start(xt[:, :, :],
                                    xT_r[:, :, j * P:(j + 1) * P])
                        return xt

                    xtq = [_xt_load(j, nc.gpsimd) for j in range(3)]
                    for j in range(NCH):
                        xt = xtq.pop(0)
                        if j + 3 < NCH:
                            xtq.append(_xt_load(j + 3, nc.sync))
                        pw = pw_pool.tile([P, W], f32, tag="pw")
                        for a in range(8):
                            nc.tensor.matmul(pw[:, :], xt[:, a, :], wa_t[a][:, :],
                                             start=(a == 0),
                                             stop=(a == 7 and not use_ba))
                        if use_ba:
                            nc.tensor.matmul(pw[:, :], onesb_s[0:1, :],
                                             ba_s[0:1, :], start=False,
                                             stop=True)
                        nc.scalar.activation(sqb_st[:, j, :], pw[:, :],
                                             AF.Square)
                        if wcopy_eng == "scalar":
                            nc.scalar.copy(w_st[:, j, :], pw[:, :])
                        else:
                            nc.vector.tensor_copy(w_st[:, j, :], pw[:, :])
                        if j == 0:
                            _late_table_loads()
                            _late_const_loads()
                        if j > 0:
                            _a_tail(j - 1)
                        if j == NCH // 2:
                            _fire_ar(nc, tc, rep, rg, fake_comm, cc1_in,
                                     cc1_out, rs_st, rt_st, rr_st,
                                     pi_st, piF, 0, cc_eng)
                        if j == 2:
                            # prefetch proj2 weights (only needed in phase B)
                            wp_t = []
                            for a in range(4):
                                t = wm.tile([P, C], bf16, tag=f"wp{a}")
                                nc.sync.dma_start(
                                    t[:, :],
                                    wpTn.ap().rearrange("(a p) n -> a p n",
                                                        p=P)[a, :, :])
                                wp_t.append(t)
                    _a_tail(NCH - 1)

                # ================= phase B: Pi, dots, y, partial proj2 =======
                with (
                    tc.tile_pool(name="piB", bufs=3) as pi_pool,
                    tc.tile_pool(name="wspB", bufs=3) as wsp_pool,
                    tc.tile_pool(name="cpeB", bufs=3) as cpe_pool,
                    tc.tile_pool(name="rdB", bufs=2) as rd3_pool,
                    tc.tile_pool(name="t1B", bufs=2) as t1_pool,
                    tc.tile_pool(name="yB", bufs=3) as y_pool,
                    tc.tile_pool(name="ytB", bufs=3) as yt_pool,
                    tc.tile_pool(name="ostB", bufs=3) as ost_pool,
                    tc.tile_pool(name="tinyB", bufs=3) as tinyB_pool,
                    tc.tile_pool(name="psp", bufs=2, space="PSUM") as psp_pool,
                    tc.tile_pool(name="pda", bufs=2, space="PSUM") as pda_pool,
                    tc.tile_pool(name="pyt", bufs=pyt_bufs, space="PSUM") as pyt_pool,
                    tc.tile_pool(name="po", bufs=po_bufs, space="PSUM") as po_pool,
                ):
                    ydeq = []

                    def _b_trans(j, y):
                        """Transpose chunk j's y (deferred two iterations so
                        the DVE y-chain has slack)."""
                        pyt = pyt_pool.tile([P, W], bf16, tag="pyt")
                        for i in range(4):
                            nc.tensor.transpose(pyt[:, i * P:(i + 1) * P],
                                                y[:, i * P:(i + 1) * P],
                                                eyeb_s[:, :])
                        yt = yt_pool.tile([P, 4, P], bf16, tag="yt")
                        yt_e = (nc.scalar.copy if yt_eng == "scalar"
                                else nc.vector.tensor_copy)
                        yt_e(yt[:, :, :],
                             pyt[:, :].rearrange("p (a q) -> p a q", q=P))
                        return yt

                    def _b_proj(j, yt):
                        """Partial proj2 for chunk j."""
                        for hh in range(2):
                            po = po_pool.tile([P, 512], f32, tag="po")
                            for a in range(4):
                                nc.tensor.matmul(
                                    po[:, :], yt[:, a, :],
                                    wp_t[a][:, hh * 512:(hh + 1) * 512],
                                    start=(a == 0),
                                    stop=(a == 3 and not use_bp))
                            if use_bp:
                                nc.tensor.matmul(
                                    po[:, :], onesb_s[0:1, :],
                                    bp_s[0:1, hh * 512:(hh + 1) * 512],
                                    start=False, stop=True)
                            ost = ost_pool.tile([P, 512], odt, tag="ost")
                            oe = ost_eng
                            if ost_eng == "split":
                                oe = "vector" if hh == 0 else "scalar"
                            elif ost_eng == "split2":
                                oe = "scalar" if hh == 0 else "vector"
                            if oe == "scalar":
                                nc.scalar.copy(ost[:, :], po[:, :])
                            else:
                                nc.vector.tensor_copy(ost[:, :], po[:, :])
                            nc.sync.dma_start(
                                out.ap()[j * P:(j + 1) * P,
                                         hh * 512:(hh + 1) * 512], ost[:, :])

                    def _b_mid(j, st):
                        """Deferred dots/y stage for chunk j (one iteration of
                        slack for the cpe->cpt chain)."""
                        pi, wsp, cpe = st
                        cpt = cpt2[j % 2]
                        bmbx_r = bmbx2[j % 2]
                        bmbx_w = bmbx2[(j + 1) % 2]
                        # D = cumA + carrA + bcast(cumPi): tri + one matmul
                        pda = pda_pool.tile([P, W], f32, tag="pda")
                        nc.tensor.matmul(pda[:, :], utb_s[:, :], wsp[:, :],
                                         start=True, stop=False)
                        nc.tensor.matmul(pda[:, :], cpt[:, :], bmbx_r[:, :],
                                         start=False, stop=True)
                        # next carrA row = total row (p0) minus its cpe part
                        nc.vector.tensor_sub(
                            bmbx_w[32:33, :].rearrange("o (h d) -> o h d", d=D),
                            pda[0:1, :].rearrange("o (h d) -> o h d", d=D),
                            cpe[0:1, :].rearrange("o (h u) -> o h u", u=1)
                            .to_broadcast((1, HL, D)))
                        rd3 = rd3_pool.tile([P, W], f32, tag="rd3")
                        nc.vector.reciprocal_approx_fast(rd3[:, :], pda[:, :])
                        g = tinyB_pool.tile([P, HL], f32, tag="g")
                        nc.vector.tensor_mul(g[:, :], pi, cpe[:, :])
                        t1 = t1_pool.tile([P, W], f32, tag="t1f")
                        t1_e = (nc.gpsimd.tensor_mul if t1_eng == "gpsimd"
                                else nc.vector.tensor_mul)
                        t1_e(t1[:, :], w_st[:, j, :], rd3[:, :])
                        y = y_pool.tile([P, W], bf16, tag="ybf")
                        nc.vector.tensor_mul(
                            y[:, :].rearrange("p (h d) -> p h d", d=D),
                            t1[:, :].rearrange("p (h d) -> p h d", d=D),
                            g[:, :].rearrange("p (h o) -> p h o", o=1)
                            .to_broadcast((P, HL, D)))
                        ydeq.append((j, y))

                    stq = []
                    ytq = []
                    for j in range(NCH):
                        if len(ydeq) == 2:
                            jt, yd = ydeq.pop(0)
                            ytq.append((jt, _b_trans(jt, yd)))
                        if len(ytq) == 2:
                            jd, ytd = ytq.pop(0)
                        else:
                            jd = None
                        pi = piF[:, j, :]
                        wsp = wsp_pool.tile([P, W], bf16, tag="wsp")
                        nc.gpsimd.tensor_mul(
                            wsp[:, :].rearrange("p (h d) -> p h d", d=D),
                            sqb_st[:, j, :].rearrange("p (h d) -> p h d", d=D),
                            pi.rearrange("p (h o) -> p h o", o=1)
                            .to_broadcast((P, HL, D)))
                        # cumPi chain (f32, exact); spare psum cols hold the
                        # transposed cpe for the cpe-broadcast matmul.
                        psp = psp_pool.tile([P, 256], f32, tag="psp")
                        nc.tensor.matmul(psp[:, 0:HL], ut_s[:, :], pi,
                                         start=True, stop=False)
                        nc.tensor.matmul(psp[:, 0:HL], onesr_s[0:1, :],
                                         carrPi[0:1, j, :], start=False,
                                         stop=True)
                        nc.scalar.copy(carrPi[0:1, j + 1, :],
                                       psp[0:1, 0:HL])
                        cpe = cpe_pool.tile([P, HL], f32, tag="cpe")
                        nc.scalar.copy(cpe[:, :], psp[:, 0:HL])
                        if jd is not None:
                            _b_proj(jd, ytd)
                        if stq:
                            _b_mid(*stq.pop(0))
                        if j == 5:
                            _fire_ar(nc, tc, rep, rg, fake_comm, cc2_in,
                                     cc2_out, rs_st, rt_st, rr_st,
                                     pi_st, piF, 1, cc_eng)
                        nc.tensor.transpose(psp[0:HL, 128:256], cpe[:, :],
                                            eye_s[:, :])
                        nc.vector.tensor_copy(cpt2[j % 2][0:HL, :],
                                              psp[0:HL, 128:256])
                        stq.append((j, (pi, wsp, cpe)))
                    _b_mid(*stq.pop(0))
                    for jt, yd in ydeq:
                        ytq.append((jt, _b_trans(jt, yd)))
                    for jd, ytd in ytq:
                        _b_proj(jd, ytd)

    nc.finalize()
    return nc


def _fire_ar(nc, tc, rep, rg, fake_comm, cc_in, cc_out, rs_st, rt_st, rr_st,
             pi_st, piF, half, cc_eng="gpsimd"):
    """Stage local 8-head exp-sums for 16 chunks, AllReduce-add with the
    partner core, read back the 16-head totals and take reciprocals."""
    import concourse.mybir as mybir
    n = NCH // 2
    lo = half * n
    nc.sync.dma_start(cc_in.ap()[rep], rs_st[:, lo:lo + n])
    if fake_comm:
        nc.sync.dma_start(cc_out.ap()[rep], cc_in.ap()[rep])
    else:
        nc.gpsimd.collective_compute(
            "AllReduce", mybir.AluOpType.add, replica_groups=rg,
            ins=[cc_in.ap()[rep].opt()], outs=[cc_out.ap()[rep].opt()])
    nc.sync.dma_start(rt_st[:, lo:lo + n], cc_out.ap()[rep])
    nc.vector.reciprocal_approx_fast(rr_st[:, lo:lo + n], rt_st[:, lo:lo + n])
    # batch-normalize: Pi = es * (1/rtot) for all 16 chunks in one DVE op
    nc.vector.tensor_mul(
        piF[:, lo:lo + n, :], pi_st[:, lo:lo + n, :],
        rr_st[:, lo:lo + n].rearrange("p (c o) -> p c o", o=1)
        .to_broadcast((P, n, HL)))


def _get_nc(**kw):
    key = tuple(sorted(kw.items()))
    if key not in _BUILD_CACHE:
        _BUILD_CACHE[key] = _build(**kw)
    return _BUILD_CACHE[key]


def make_in_maps(x, Wa, ba, Wp, bp, temp, denom_bias):
    """Host-side sharding: core i -> (b=i//2, head-half=i%2)."""
    bf = ml_dtypes.bfloat16
    waT = np.ascontiguousarray(Wa.T).astype(bf)          # [C, C]
    wpTn = np.ascontiguousarray(-Wp.T).astype(bf)        # [C, C]
    # token rows reversed within each 128-chunk (see _build)
    xTs = []
    for b in range(B):
        xr = x[b].reshape(NCH, P, C)[:, ::-1, :].reshape(T, C)
        xTs.append(np.ascontiguousarray(xr.T).astype(bf))
    in_maps = []
    for i in range(N_CORES):
        b, hh = i // 2, i % 2
        wa_loc = np.ascontiguousarray(waT[:, hh * W:(hh + 1) * W])
        wp_loc = np.ascontiguousarray(wpTn[hh * W:(hh + 1) * W, :])
        ba_loc = np.ascontiguousarray(
            ba[hh * W:(hh + 1) * W].reshape(1, W)).astype(bf)
        bp_half = np.ascontiguousarray((bp / 2.0).reshape(1, C)).astype(bf)
        tb = np.ascontiguousarray(np.broadcast_to(
            temp[hh * HL:(hh + 1) * HL].reshape(1, HL), (P, HL))
        ).astype(np.float32)
        dbr = (D * denom_bias[hh * HL:(hh + 1) * HL, :, 0].T)
        dbr = dbr.reshape(NCH, P, HL)[:, ::-1, :].reshape(T, HL)
        db64 = np.ascontiguousarray(dbr).astype(np.float32)
        in_maps.append({
            "xT": xTs[b], "waT": wa_loc, "wpTn": wp_loc, "ba": ba_loc,
            "bp": bp_half, "tb": tb, "db64": db64,
        })
    return in_maps


def kernel(x, Wa, ba, Wp, bp, temp, denom_bias):
    x = np.asarray(x)
    use_ba = bool(np.any(np.asarray(ba)))
    use_bp = bool(np.any(np.asarray(bp)))
    use_tmpscale = bool(np.any(np.asarray(denom_bias))
                        or not np.all(np.asarray(temp) == 1.0))
    nc = _get_nc(use_ba=use_ba, use_bp=use_bp, use_tmpscale=use_tmpscale)
    in_maps = make_in_maps(np.asarray(x), np.asarray(Wa), np.asarray(ba),
                           np.asarray(Wp), np.asarray(bp), np.asarray(temp),
                           np.asarray(denom_bias))
    from concourse import bass_utils
    res = bass_utils.run_bass_kernel_spmd(nc, in_maps,
                                          core_ids=list(range(N_CORES)))
    out = np.empty((B, T, C), np.float32)
    for b in range(B):
        s = (res.results[2 * b]["out"].astype(np.float32)
             + res.results[2 * b + 1]["out"].astype(np.float32))
        out[b] = s.reshape(NCH, P, C)[:, ::-1, :].reshape(T, C)
    return out


# revision 4
# speedup vs baseline: 1.0302x; 1.0192x over previous
"""TSSA causal self-attention Bass kernel, v2: head-split sharding.

Sharding: core i -> (batch b = i//2, head-half hh = i%2, i.e. heads
hh*8..hh*8+7 = w columns [hh*512:(hh+1)*512]).  Each core runs the FULL
T=4096 for its 8 heads, so the causal cumsums are core-local (carry chained
through the triangular matmul's last PSUM row -- no cross-core carry, no
one-hot chunk-sum matmuls, no prefix tables).

The only cross-core data dependency is the softmax-over-heads denominator:
rtot[t] = sum over all 16 heads of exp(tmp).  Each core AllReduce-adds its
local 8-head partial sums (two 8KB collectives, each fired 16 chunks before
its consumer -> fully hidden behind compute).

proj2 is column-split: each core computes a PARTIAL output
y_local @ (-Wp.T)[local 512 rows, :] over the full (T, C) and the host adds
the two partials per batch.  No y exchange (collectives are ~15us overhead +
40GB/s -- a 2MB exchange would cost ~67us).

Math per (b, local heads):
    w      = x @ Wa.T[:, loc]            # (T, 512) -> heads (T, 8, 64)
    wsq    = w * w
    denom  = cumsum_T(wsq) + 1e-12       # chained tri matmuls
    tmp    = seghsum_d(wsq / denom)      # fused DVE scan + boundary diff
    es     = exp(tmp)                    # (T, 8); rs = sum_h es
    Pi     = es / AllReduceAdd(rs)       # softmax over all 16 heads
    cumPi  = cumsum_T(Pi) + 1e-8         # f32 chain (exact)
    D      = cumsum_T(wsq*Pi) + cumPi    # cumA chain + cpe bcast via matmul
    y      = (w * Pi * cumPi) * (1/D)    # sign folded into wpTn
    out_p  = y @ (-Wp.T)[loc, :]         # partial; host adds the two halves
"""

import numpy as np
import ml_dtypes

B, T, C, H, D = 4, 4096, 1024, 16, 64
N_CORES = 8
P = 128
HL = H // 2          # 8 local heads
W = HL * D           # 512 local w columns
NCH = T // P         # 32 chunks

F32 = None
BF16 = None

_BUILD_CACHE = {}


def _ensure_scan_op():
    """Register a custom DVE op: inclusive prefix-scan of Src0*Src1 along the
    free dim.  Per-head sums then come from differencing the scan at head
    boundaries, fusing the (mul, segmented-reduce) pair into one DVE pass."""
    from concourse import dve_ops as dops
    if hasattr(dops, "_TT_MUL_SCAN_ANT"):
        return dops._TT_MUL_SCAN_ANT
    import numpy as np
    from concourse.dve_spec import Spec, Src0, Src1, AluOp, Bin, scan, lower, _has_src1
    from concourse.dve_uop import DveOpSpec

    def _ref(in0, in1, s0, s1, imm2):
        return np.cumsum(in0.astype(np.float32) * in1.astype(np.float32),
                         axis=-1, dtype=np.float32)

    spec = Spec(body=scan(AluOp.ADD, Bin(AluOp.MULTIPLY, Src0, Src1)),
                reference=_ref)
    shas = {}
    for ver in ("v3", "v4"):
        try:
            r = DveOpSpec(name="TT_MUL_SCAN_ANT", uops=lower(spec, ver=ver),
                          rd1_en=_has_src1(spec))
            shas[ver] = r.sha(ver)
        except Exception:
            pass
    op = dops.DveOp("TT_MUL_SCAN_ANT", spec, subdim=False, uops_sha=shas)
    dops.OPS.append(op)
    dops.CUSTOM_DVE_SPECS[op.name] = spec
    dops._SUB_OPCODE_FOR_NAME[op.name] = max(dops._SUB_OPCODE_FOR_NAME.values()) + 1
    dops._TT_MUL_SCAN_ANT = op
    return op


def _build(n_groups=4, fake_comm=False, use_ba=False, use_bp=False,
           use_tmpscale=False, f32out=False, reps=1,
           wcopy_eng="vector", yt_eng="scalar", pw_bufs=4, pdn_bufs=2,
           ost_eng="scalar", t1_eng="vector", pyt_bufs=2, po_bufs=2,
           cc_eng="gpsimd"):
    """Build the SPMD Bass program (symmetric across all cores)."""
    import concourse.bass as bass
    import concourse.bacc as bacc
    import concourse.mybir as mybir
    from concourse import tile

    dt = mybir.dt
    f32, bf16 = dt.float32, dt.bfloat16
    AF = mybir.ActivationFunctionType

    odt = f32 if f32out else bf16

    scan_op = _ensure_scan_op()

    nc = bacc.Bacc(None, target_bir_lowering=False, debug=False)

    # ---------------- I/O ----------------
    xT = nc.dram_tensor("xT", [C, T], bf16, kind="ExternalInput")
    waT = nc.dram_tensor("waT", [C, W], bf16, kind="ExternalInput")
    wpTn = nc.dram_tensor("wpTn", [W, C], bf16, kind="ExternalInput")
    ba_in = nc.dram_tensor("ba", [1, W], bf16, kind="ExternalInput")
    bp_in = nc.dram_tensor("bp", [1, C], bf16, kind="ExternalInput")
    tb_in = nc.dram_tensor("tb", [P, HL], f32, kind="ExternalInput")
    db_in = nc.dram_tensor("db64", [T, HL], f32, kind="ExternalInput")
    out = nc.dram_tensor("out", [T, C], odt, kind="ExternalOutput")

    # constants baked into the NEFF.  Token rows are REVERSED within each
    # 128-chunk (host pre-flips xT), so the causal cumsum is a LOWER-tri
    # matmul and the chunk total lands on PSUM partition 0 -- the carry
    # chain is then all partition-0 copies.
    lt_np = np.tril(np.ones((P, P), np.float32))
    utb_c = nc.inline_tensor(lt_np.astype(ml_dtypes.bfloat16), "utb_c")
    ut_c = nc.inline_tensor(lt_np, "ut_c")
    eye_np = np.eye(P, dtype=np.float32)
    eyeb_c = nc.inline_tensor(eye_np.astype(ml_dtypes.bfloat16), "eyeb_c")
    eye_c = nc.inline_tensor(eye_np, "eye_c")
    onesb_c = nc.inline_tensor(np.ones((1, P), ml_dtypes.bfloat16), "onesb_c")
    onesr_c = nc.inline_tensor(np.ones((1, P), np.float32), "onesr_c")
    # bmbx: rows 0..7 = head selector rows for the cpe-broadcast matmul,
    # rows 8..31 zero, row 32 = dynamic carrA row (32-aligned partition base
    # so the DVE chain-update may write it).
    bm_np = np.zeros((33, W), np.float32)
    for h in range(HL):
        bm_np[h, h * D:(h + 1) * D] = 1.0
    bmb_c = nc.inline_tensor(bm_np.astype(ml_dtypes.bfloat16), "bmb_c")
    # cpt: rows 0..7 = cpe^T (per chunk), row 32 = ones; zero-init the rest
    # (junk x 0 would still contaminate the matmul if it were NaN).
    cpt_np = np.zeros((33, P), np.float32)
    cpt_np[32, :] = 1.0
    cpt_c = nc.inline_tensor(cpt_np.astype(ml_dtypes.bfloat16), "cpt_c")
    carr0_c = nc.inline_tensor(
        np.full((1, W), 1e-12, ml_dtypes.bfloat16), "carr0_c")
    carrPi0_c = nc.inline_tensor(np.full((1, HL), 1e-8, np.float32),
                                 "carrPi0_c")

    # internal DRAM for the two softmax-denominator AllReduces
    cc1_in = nc.dram_tensor("cc1_in", [reps, P, NCH // 2], f32, kind="Internal")
    cc1_out = nc.dram_tensor("cc1_out", [reps, P, NCH // 2], f32,
                             kind="Internal")
    cc2_in = nc.dram_tensor("cc2_in", [reps, P, NCH // 2], f32, kind="Internal")
    cc2_out = nc.dram_tensor("cc2_out", [reps, P, NCH // 2], f32,
                             kind="Internal")
    rg = [[2 * g, 2 * g + 1] for g in range(n_groups)]

    with tile.TileContext(nc) as tc:
        with (
            tc.tile_pool(name="const", bufs=1) as cpool,
            tc.tile_pool(name="persist", bufs=1) as pp,
            tc.tile_pool(name="wmat", bufs=1) as wm,
        ):
            for rep in range(reps):
                # ---- phase-A-critical loads first ----
                wa_t = []
                for a in range(8):
                    t = wm.tile([P, W], bf16, tag=f"wa{a}")
                    nc.sync.dma_start(
                        t[:, :],
                        waT.ap().rearrange("(a p) n -> a p n", p=P)[a, :, :])
                    wa_t.append(t)
                utb_s = cpool.tile([P, P], bf16, tag="utb")
                nc.sync.dma_start(utb_s[:, :], utb_c.ap())
                onesb_s = cpool.tile([1, P], bf16, tag="onesb")
                nc.sync.dma_start(onesb_s[:, :], onesb_c.ap())
                ba_s = cpool.tile([1, W], bf16, tag="ba")
                if use_ba:
                    nc.sync.dma_start(ba_s[:, :], ba_in.ap())
                ut_s = cpool.tile([P, P], f32, tag="ut")
                eyeb_s = cpool.tile([P, P], bf16, tag="eyeb")
                eye_s = cpool.tile([P, P], f32, tag="eye")
                onesr_s = cpool.tile([1, P], f32, tag="onesr")
                bp_s = cpool.tile([1, C], bf16, tag="bp")
                tb_s = cpool.tile([P, HL], f32, tag="tb")
                db_s = cpool.tile([P, NCH, HL], f32, tag="db")

                def _late_const_loads():
                    nc.sync.dma_start(ut_s[:, :], ut_c.ap())
                    nc.sync.dma_start(eyeb_s[:, :], eyeb_c.ap())
                    nc.sync.dma_start(eye_s[:, :], eye_c.ap())
                    nc.sync.dma_start(onesr_s[:, :], onesr_c.ap())
                    if use_bp:
                        nc.sync.dma_start(bp_s[:, :], bp_in.ap())
                    if use_tmpscale:
                        nc.sync.dma_start(tb_s[:, :], tb_in.ap())
                        nc.sync.dma_start(
                            db_s[:, :, :],
                            db_in.ap().rearrange("(j p) h -> p j h", p=P))

                # ---- persistent stores ----
                w_st = pp.tile([P, NCH, W], bf16, tag="w_st")
                sqb_st = pp.tile([P, NCH, W], bf16, tag="sqb_st")
                pi_st = pp.tile([P, NCH, HL], f32, tag="pi_st")   # holds es
                piF = pp.tile([P, NCH, HL], f32, tag="piF")
                rs_st = pp.tile([P, NCH], f32, tag="rs_st")
                rt_st = pp.tile([P, NCH], f32, tag="rt_st")
                rr_st = pp.tile([P, NCH], f32, tag="rr_st")
                carrb = pp.tile([1, NCH + 1, W], bf16, tag="carrb")
                carrPi = pp.tile([1, NCH + 1, HL], f32, tag="carrPi")
                bmbx2, cpt2 = [], []
                for p_ in range(2):
                    bmbx_p = pp.tile([33, W], bf16, tag=f"bmbx{p_}")
                    bmbx2.append(bmbx_p)
                    cpt_p = pp.tile([33, P], bf16, tag=f"cpt{p_}")
                    cpt2.append(cpt_p)

                def _late_table_loads():
                    nc.sync.dma_start(carrb[0:1, 0, :], carr0_c.ap())
                    nc.sync.dma_start(carrPi[0:1, 0, :], carrPi0_c.ap())
                    for p_ in range(2):
                        nc.sync.dma_start(bmbx2[p_][:, :], bmb_c.ap())
                        nc.sync.dma_start(cpt2[p_][:, :], cpt_c.ap())

                xT_r = xT.ap().rearrange("(a p) t -> p a t", p=P)

                # ================= phase A: proj1, denom, softmax numerator ==
                with (
                    tc.tile_pool(name="xt", bufs=3) as xt_pool,
                    tc.tile_pool(name="rdA", bufs=2) as rd_pool,
                    tc.tile_pool(name="wnA", bufs=2) as wn_pool,
                    tc.tile_pool(name="tinyA", bufs=3) as tiny_pool,
                    tc.tile_pool(name="pw", bufs=pw_bufs,
                                 space="PSUM") as pw_pool,
                    tc.tile_pool(name="pdn", bufs=pdn_bufs,
                                 space="PSUM") as pdn_pool,
                ):
                    def _a_tri(j, pdn2):
                        """Tri + chained carry for chunk j into bank j%2 of
                        the pair tile; extract next carry from partition 0."""
                        h = j % 2
                        nc.tensor.matmul(pdn2[:, h, :], utb_s[:, :],
                                         sqb_st[:, j, :], start=True,
                                         stop=False)
                        nc.tensor.matmul(pdn2[:, h, :], onesb_s[0:1, :],
                                         carrb[0:1, j, :], start=False,
                                         stop=True)
                        nc.scalar.copy(carrb[0:1, j + 1, :], pdn2[0:1, h, :])

                    def _a_pair_tail(p, pdn2):
                        """Fused DVE stage for chunks 2p, 2p+1: one recip and
                        one scan over both banks; the boundary diff at head 8
                        subtracts chunk 2p's running total, which is exactly
                        the segmented sum for chunk 2p+1's head 0."""
                        j0 = 2 * p
                        rd = rd_pool.tile([P, 2, W], f32, tag="rdA")
                        nc.vector.reciprocal_approx_fast(rd[:, :, :],
                                                         pdn2[:, :, :])
                        wn = wn_pool.tile([P, 2, W], f32, tag="wnA")
                        nc.vector._custom_dve(scan_op, out=wn[:, :, :],
                                              in0=sqb_st[:, j0:j0 + 2, :],
                                              in1=rd[:, :, :])
                        wn3 = wn[:, :, :].rearrange("p g (h d) -> p (g h) d",
                                                    d=D)
                        red = tiny_pool.tile([P, 2 * HL], f32, tag="red")
                        nc.vector.tensor_copy(red[:, 0:1], wn3[:, 0:1, D - 1])
                        nc.vector.tensor_sub(
                            red[:, 1:2 * HL].rearrange("p (h o) -> p h o", o=1),
                            wn3[:, 1:2 * HL, D - 1],
                            wn3[:, 0:2 * HL - 1, D - 1])
                        if use_tmpscale:
                            t1 = tiny_pool.tile([P, 2 * HL], f32, tag="t1")
                            nc.vector.tensor_add(
                                t1[:, :], red[:, :],
                                db_s[:, j0:j0 + 2, :]
                                .rearrange("p g h -> p (g h)"))
                            tmpj = tiny_pool.tile([P, 2 * HL], f32, tag="tmpj")
                            nc.vector.tensor_mul(
                                tmpj[:, :].rearrange("p (g h) -> p g h", h=HL),
                                t1[:, :].rearrange("p (g h) -> p g h", h=HL),
                                tb_s[:, :].rearrange("p (g h) -> p g h", g=1)
                                .to_broadcast((P, 2, HL)))
                        else:
                            tmpj = red
                        # tmp <= D*temp = 64: exp fits f32, skip max-sub
                        for g in range(2):
                            nc.scalar.activation(
                                pi_st[:, j0 + g, :],
                                tmpj[:, g * HL:(g + 1) * HL], AF.Exp,
                                accum_out=rs_st[:, j0 + g:j0 + g + 1])

                    def _xt_load(j, q):
                        xt = xt_pool.tile([P, 8, P], bf16, tag="xt")
                        q.dma_start(xt[:, :, :],
                                    xT_r[:, :, j * P:(j + 1) * P])
                        return xt

                    xtq = [_xt_load(j, nc.gpsimd) for j in range(3)]
                    pdn2 = None
                    for j in range(NCH):
                        xt = xtq.pop(0)
                        if j + 3 < NCH:
                            xtq.append(_xt_load(j + 3, nc.sync))
                        pw = pw_pool.tile([P, W], f32, tag="pw")
                        for a in range(8):
                            nc.tensor.matmul(pw[:, :], xt[:, a, :], wa_t[a][:, :],
                                             start=(a == 0),
                                             stop=(a == 7 and not use_ba))
                        if use_ba:
                            nc.tensor.matmul(pw[:, :], onesb_s[0:1, :],
                                             ba_s[0:1, :], start=False,
                                             stop=True)
                        nc.scalar.activation(sqb_st[:, j, :], pw[:, :],
                                             AF.Square)
                        if wcopy_eng == "scalar":
                            nc.scalar.copy(w_st[:, j, :], pw[:, :])
                        else:
                            nc.vector.tensor_copy(w_st[:, j, :], pw[:, :])
                        if j == 0:
                            _late_table_loads()
                            _late_const_loads()
                        if j > 0:
                            jj = j - 1
                            if jj % 2 == 0:
                                pdn2 = pdn_pool.tile([P, 2, W], f32,
                                                     tag="pdn")
                            _a_tri(jj, pdn2)
                            if jj % 2 == 1:
                                _a_pair_tail(jj // 2, pdn2)
                        if j == NCH // 2:
                            _fire_ar(nc, tc, rep, rg, fake_comm, cc1_in,
                                     cc1_out, rs_st, rt_st, rr_st,
                                     pi_st, piF, 0, cc_eng)
                        if j == 2:
                            # prefetch proj2 weights (only needed in phase B)
                            wp_t = []
                            for a in range(4):
                                t = wm.tile([P, C], bf16, tag=f"wp{a}")
                                nc.sync.dma_start(
                                    t[:, :],
                                    wpTn.ap().rearrange("(a p) n -> a p n",
                                                        p=P)[a, :, :])
                                wp_t.append(t)
                    _a_tri(NCH - 1, pdn2)
                    _a_pair_tail(NCH // 2 - 1, pdn2)

                # ================= phase B: Pi, dots, y, partial proj2 =======
                with (
                    tc.tile_pool(name="piB", bufs=3) as pi_pool,
                    tc.tile_pool(name="wspB", bufs=3) as wsp_pool,
                    tc.tile_pool(name="cpeB", bufs=3) as cpe_pool,
                    tc.tile_pool(name="rdB", bufs=2) as rd3_pool,
                    tc.tile_pool(name="t1B", bufs=2) as t1_pool,
                    tc.tile_pool(name="yB", bufs=3) as y_pool,
                    tc.tile_pool(name="ytB", bufs=3) as yt_pool,
                    tc.tile_pool(name="ostB", bufs=3) as ost_pool,
                    tc.tile_pool(name="tinyB", bufs=3) as tinyB_pool,
                    tc.tile_pool(name="psp", bufs=2, space="PSUM") as psp_pool,
                    tc.tile_pool(name="pda", bufs=2, space="PSUM") as pda_pool,
                    tc.tile_pool(name="pyt", bufs=pyt_bufs, space="PSUM") as pyt_pool,
                    tc.tile_pool(name="po", bufs=po_bufs, space="PSUM") as po_pool,
                ):
                    ydeq = []

                    def _b_trans(j, y):
                        """Transpose chunk j's y (deferred two iterations so
                        the DVE y-chain has slack)."""
                        pyt = pyt_pool.tile([P, W], bf16, tag="pyt")
                        for i in range(4):
                            nc.tensor.transpose(pyt[:, i * P:(i + 1) * P],
                                                y[:, i * P:(i + 1) * P],
                                                eyeb_s[:, :])
                        yt = yt_pool.tile([P, 4, P], bf16, tag="yt")
                        yt_e = (nc.scalar.copy if yt_eng == "scalar"
                                else nc.vector.tensor_copy)
                        yt_e(yt[:, :, :],
                             pyt[:, :].rearrange("p (a q) -> p a q", q=P))
                        return yt

                    def _b_proj(j, yt):
                        """Partial proj2 for chunk j."""
                        for hh in range(2):
                            po = po_pool.tile([P, 512], f32, tag="po")
                            for a in range(4):
                                nc.tensor.matmul(
                                    po[:, :], yt[:, a, :],
                                    wp_t[a][:, hh * 512:(hh + 1) * 512],
                                    start=(a == 0),
                                    stop=(a == 3 and not use_bp))
                            if use_bp:
                                nc.tensor.matmul(
                                    po[:, :], onesb_s[0:1, :],
                                    bp_s[0:1, hh * 512:(hh + 1) * 512],
                                    start=False, stop=True)
                            ost = ost_pool.tile([P, 512], odt, tag="ost")
                            oe = ost_eng
                            if ost_eng == "split":
                                oe = "vector" if hh == 0 else "scalar"
                            elif ost_eng == "split2":
                                oe = "scalar" if hh == 0 else "vector"
                            if oe == "scalar":
                                nc.scalar.copy(ost[:, :], po[:, :])
                            else:
                                nc.vector.tensor_copy(ost[:, :], po[:, :])
                            nc.sync.dma_start(
                                out.ap()[j * P:(j + 1) * P,
                                         hh * 512:(hh + 1) * 512], ost[:, :])

                    def _b_mid(j, st):
                        """Deferred dots/y stage for chunk j (one iteration of
                        slack for the cpe->cpt chain)."""
                        pi, wsp, cpe = st
                        cpt = cpt2[j % 2]
                        bmbx_r = bmbx2[j % 2]
                        bmbx_w = bmbx2[(j + 1) % 2]
                        # D = cumA + carrA + bcast(cumPi): tri + one matmul
                        pda = pda_pool.tile([P, W], f32, tag="pda")
                        nc.tensor.matmul(pda[:, :], utb_s[:, :], wsp[:, :],
                                         start=True, stop=False)
                        nc.tensor.matmul(pda[:, :], cpt[:, :], bmbx_r[:, :],
                                         start=False, stop=True)
                        # next carrA row = total row (p0) minus its cpe part
                        nc.vector.tensor_sub(
                            bmbx_w[32:33, :].rearrange("o (h d) -> o h d", d=D),
                            pda[0:1, :].rearrange("o (h d) -> o h d", d=D),
                            cpe[0:1, :].rearrange("o (h u) -> o h u", u=1)
                            .to_broadcast((1, HL, D)))
                        rd3 = rd3_pool.tile([P, W], f32, tag="rd3")
                        nc.vector.reciprocal_approx_fast(rd3[:, :], pda[:, :])
                        g = tinyB_pool.tile([P, HL], f32, tag="g")
                        nc.vector.tensor_mul(g[:, :], pi, cpe[:, :])
                        t1 = t1_pool.tile([P, W], f32, tag="t1f")
                        t1_e = (nc.gpsimd.tensor_mul if t1_eng == "gpsimd"
                                else nc.vector.tensor_mul)
                        t1_e(t1[:, :], w_st[:, j, :], rd3[:, :])
                        y = y_pool.tile([P, W], bf16, tag="ybf")
                        nc.vector.tensor_mul(
                            y[:, :].rearrange("p (h d) -> p h d", d=D),
                            t1[:, :].rearrange("p (h d) -> p h d", d=D),
                            g[:, :].rearrange("p (h o) -> p h o", o=1)
                            .to_broadcast((P, HL, D)))
                        ydeq.append((j, y))

                    stq = []
                    ytq = []
                    for j in range(NCH):
                        if len(ydeq) == 2:
                            jt, yd = ydeq.pop(0)
                            ytq.append((jt, _b_trans(jt, yd)))
                        if len(ytq) == 2:
                            jd, ytd = ytq.pop(0)
                        else:
                            jd = None
                        pi = piF[:, j, :]
                        wsp = wsp_pool.tile([P, W], bf16, tag="wsp")
                        nc.gpsimd.tensor_mul(
                            wsp[:, :].rearrange("p (h d) -> p h d", d=D),
                            sqb_st[:, j, :].rearrange("p (h d) -> p h d", d=D),
                            pi.rearrange("p (h o) -> p h o", o=1)
                            .to_broadcast((P, HL, D)))
                        # cumPi chain (f32, exact); spare psum cols hold the
                        # transposed cpe for the cpe-broadcast matmul.
                        psp = psp_pool.tile([P, 256], f32, tag="psp")
                        nc.tensor.matmul(psp[:, 0:HL], ut_s[:, :], pi,
                                         start=True, stop=False)
                        nc.tensor.matmul(psp[:, 0:HL], onesr_s[0:1, :],
                                         carrPi[0:1, j, :], start=False,
                                         stop=True)
                        nc.scalar.copy(carrPi[0:1, j + 1, :],
                                       psp[0:1, 0:HL])
                        cpe = cpe_pool.tile([P, HL], f32, tag="cpe")
                        nc.scalar.copy(cpe[:, :], psp[:, 0:HL])
                        if jd is not None:
                            _b_proj(jd, ytd)
                        if stq:
                            _b_mid(*stq.pop(0))
                        if j == 5:
                            _fire_ar(nc, tc, rep, rg, fake_comm, cc2_in,
                                     cc2_out, rs_st, rt_st, rr_st,
                                     pi_st, piF, 1, cc_eng)
                        nc.tensor.transpose(psp[0:HL, 128:256], cpe[:, :],
                                            eye_s[:, :])
                        nc.vector.tensor_copy(cpt2[j % 2][0:HL, :],
                                              psp[0:HL, 128:256])
                        stq.append((j, (pi, wsp, cpe)))
                    _b_mid(*stq.pop(0))
                    for jt, yd in ydeq:
                        ytq.append((jt, _b_trans(jt, yd)))
                    for jd, ytd in ytq:
                        _b_proj(jd, ytd)

    nc.finalize()
    return nc


def _fire_ar(nc, tc, rep, rg, fake_comm, cc_in, cc_out, rs_st, rt_st, rr_st,
             pi_st, piF, half, cc_eng="gpsimd"):
    """Stage local 8-head exp-sums for 16 chunks, AllReduce-add with the
    partner core, read back the 16-head totals and take reciprocals."""
    import concourse.mybir as mybir
    n = NCH // 2
    lo = half * n
    nc.sync.dma_start(cc_in.ap()[rep], rs_st[:, lo:lo + n])
    if fake_comm:
        nc.sync.dma_start(cc_out.ap()[rep], cc_in.ap()[rep])
    else:
        nc.gpsimd.collective_compute(
            "AllReduce", mybir.AluOpType.add, replica_groups=rg,
            ins=[cc_in.ap()[rep].opt()], outs=[cc_out.ap()[rep].opt()])
    nc.sync.dma_start(rt_st[:, lo:lo + n], cc_out.ap()[rep])
    nc.vector.reciprocal_approx_fast(rr_st[:, lo:lo + n], rt_st[:, lo:lo + n])
    # batch-normalize: Pi = es * (1/rtot) for all 16 chunks in one DVE op
    nc.vector.tensor_mul(
        piF[:, lo:lo + n, :], pi_st[:, lo:lo + n, :],
        rr_st[:, lo:lo + n].rearrange("p (c o) -> p c o", o=1)
        .to_broadcast((P, n, HL)))


def _get_nc(**kw):
    key = tuple(sorted(kw.items()))
    if key not in _BUILD_CACHE:
        _BUILD_CACHE[key] = _build(**kw)
    return _BUILD_CACHE[key]


def make_in_maps(x, Wa, ba, Wp, bp, temp, denom_bias):
    """Host-side sharding: core i -> (b=i//2, head-half=i%2)."""
    bf = ml_dtypes.bfloat16
    waT = np.ascontiguousarray(Wa.T).astype(bf)          # [C, C]
    wpTn = np.ascontiguousarray(-Wp.T).astype(bf)        # [C, C]
    # token rows reversed within each 128-chunk (see _build)
    xTs = []
    for b in range(B):
        xr = x[b].reshape(NCH, P, C)[:, ::-1, :].reshape(T, C)
        xTs.append(np.ascontiguousarray(xr.T).astype(bf))
    in_maps = []
    for i in range(N_CORES):
        b, hh = i // 2, i % 2
        wa_loc = np.ascontiguousarray(waT[:, hh * W:(hh + 1) * W])
        wp_loc = np.ascontiguousarray(wpTn[hh * W:(hh + 1) * W, :])
        ba_loc = np.ascontiguousarray(
            ba[hh * W:(hh + 1) * W].reshape(1, W)).astype(bf)
        bp_half = np.ascontiguousarray((bp / 2.0).reshape(1, C)).astype(bf)
        tb = np.ascontiguousarray(np.broadcast_to(
            temp[hh * HL:(hh + 1) * HL].reshape(1, HL), (P, HL))
        ).astype(np.float32)
        dbr = (D * denom_bias[hh * HL:(hh + 1) * HL, :, 0].T)
        dbr = dbr.reshape(NCH, P, HL)[:, ::-1, :].reshape(T, HL)
        db64 = np.ascontiguousarray(dbr).astype(np.float32)
        in_maps.append({
            "xT": xTs[b], "waT": wa_loc, "wpTn": wp_loc, "ba": ba_loc,
            "bp": bp_half, "tb": tb, "db64": db64,
        })
    return in_maps


def kernel(x, Wa, ba, Wp, bp, temp, denom_bias):
    x = np.asarray(x)
    use_ba = bool(np.any(np.asarray(ba)))
    use_bp = bool(np.any(np.asarray(bp)))
    use_tmpscale = bool(np.any(np.asarray(denom_bias))
                        or not np.all(np.asarray(temp) == 1.0))
    nc = _get_nc(use_ba=use_ba, use_bp=use_bp, use_tmpscale=use_tmpscale)
    in_maps = make_in_maps(np.asarray(x), np.asarray(Wa), np.asarray(ba),
                           np.asarray(Wp), np.asarray(bp), np.asarray(temp),
                           np.asarray(denom_bias))
    from concourse import bass_utils
    res = bass_utils.run_bass_kernel_spmd(nc, in_maps,
                                          core_ids=list(range(N_CORES)))
    out = np.empty((B, T, C), np.float32)
    for b in range(B):
        s = (res.results[2 * b]["out"].astype(np.float32)
             + res.results[2 * b + 1]["out"].astype(np.float32))
        out[b] = s.reshape(NCH, P, C)[:, ::-1, :].reshape(T, C)
    return out
